# revision 27
# baseline (speedup 1.0000x reference)
"""Trainium2 Bass kernel for causal cross-attention with L2-normalized q/k.

Reference computation (B=4, S=2048, E=512, H=8, Dh=64):
    q = k_embed @ Wq.T ; k = x @ Wk.T ; v = x @ Wv.T        (per batch)
    q,k l2-normalized over Dh per head; scores = g * q @ k.T (causal mask)
    out = softmax(scores) @ v

Sharding: 8 cores = 4 batches x 2 head-groups (4 heads each).

Per-core pipeline (all matmuls bf16, PSUM accumulation fp32):
  - inputs DMA'd as bf16 (host casts), streamed in s-range chunks so
    projections start early
  - projections in natural layout, kn/qn stored pair-major; v copied to SBUF
    by ACT, kn/qn by DVE; squared norms via fused DVE tensor_tensor_reduce;
    sqrt on ACT per half; normalize muls split DVE (pair 0) / GPSIMD (pair 1);
    q/k transposed with batched DMA-engine xbar transposes (8 total)
  - attention per (j-block of 512 sq, head): scores transposed st[sk, sq]
    (K=64), two k-tiles per PSUM tile; one merged exp (ACT, scale=g) per
    i-pair into an SBUF et block buffer (bf16); causal diagonal via 128x128
    keep-mask mul (DVE)
  - PV in natural layout, c-outer/i-inner: lhsT = et 128x128 chunk
    (stationary), moving operand = v (65 cols: 64 v-cols + ones col for the
    softmax denominator); accumulated in a 1-bank PSUM tile [sq_chunk, 65]
  - out staged via DVE to SBUF, DMA'd as [S, 4, 65]; host divides by the
    denominator column and concatenates slices.
"""

import numpy as np

B, S, E, H = 4, 2048, 512, 8
Dh = 64
NE = E // 128           # 4 contraction chunks
NT = S // 128           # 16 s-tiles
SQB = 512               # sq block width
NJ = S // SQB           # 4
NC = SQB // 128         # 4 sq chunks per block
NH = NT // 2            # tiles per half


def _build(g: float, repeats: int = 1, stage: int = 7):
    from contextlib import ExitStack

    import concourse.tile as tile
    from concourse import bacc, mybir

    f32 = mybir.dt.float32
    bf16 = mybir.dt.bfloat16
    AF = mybir.ActivationFunctionType
    ALU = mybir.AluOpType

    nc = bacc.Bacc("TRN2", target_bir_lowering=False, debug=False)
    # Pin the ACT function-set choice to a table containing every function
    # this kernel uses (copy/ln/exp), so exactly one table load is emitted.
    import types

    import bass_rust as _br
    from concourse.hw_specs import get_activation_tables

    def _insert_act_table_loads(self):
        has_activation = any(
            isinstance(i, mybir.InstActivation)
            for b in self.main_func.blocks
            for i in b.instructions
        )
        if not has_activation:
            return
        tables = list(get_activation_tables(self.m.arch).items())
        need = {mybir.ActivationFunctionType.Copy,
                mybir.ActivationFunctionType.Ln,
                mybir.ActivationFunctionType.Exp}
        # act_func_set_id indexes the ORIGINAL act_info.json order, so keep
        # list positions and instead blank out every non-preferred table;
        # the pass then has a single candidate serving copy/ln/exp.
        tables = [(name, (s if need <= s else set())) for name, s in tables]
        _br.insert_act_table_loads(self, tables)

    nc.insert_act_table_loads = types.MethodType(_insert_act_table_loads, nc)
    xT_d = nc.dram_tensor("xt", [E, S], bf16, kind="ExternalInput")
    keT_d = nc.dram_tensor("ket", [E, S], bf16, kind="ExternalInput")
    wqT_d = nc.dram_tensor("wqt", [E, 256], bf16, kind="ExternalInput")
    wvkT_d = nc.dram_tensor("wvkt", [E, 512], bf16, kind="ExternalInput")
    mask_d = nc.dram_tensor("mask01", [128, 128], bf16, kind="ExternalInput")
    out_d = nc.dram_tensor("outn", [S, 4, 65], f32, kind="ExternalOutput")

    xT_r = xT_d.rearrange("(c p) m -> p c m", p=128)
    keT_r = keT_d.rearrange("(c p) m -> p c m", p=128)
    out_r = out_d.rearrange("(j c p) h e -> j p c h e", j=NJ, c=NC, p=128)

    with tile.TileContext(nc) as tc:
     for _rep in range(repeats):
      with ExitStack() as ctx:
        persist = ctx.enter_context(tc.tile_pool(name=f"persist{_rep}", bufs=1))
        sqp = ctx.enter_context(tc.tile_pool(name=f"sqp{_rep}", bufs=2))
        etp = ctx.enter_context(tc.tile_pool(name=f"etp{_rep}", bufs=2))
        osb = ctx.enter_context(tc.tile_pool(name=f"osb{_rep}", bufs=2))

        # ---- persistent SBUF ----
        wq_sb = persist.tile([128, NE, 256], bf16, tag="wq")
        wvk_sb = persist.tile([128, NE, 512], bf16, tag="wvk")
        mask_sb = persist.tile([128, 128], bf16, tag="mask")
        x_sb = persist.tile([128, NE, S], bf16, tag="x")
        ke_sb = persist.tile([128, NE, S], bf16, tag="ke")
        v_sb = persist.tile([128, NT, 4, 65], bf16, tag="v")
        # pair-major: [s_local, pair, s_tile, dh(2 heads x 64)]
        kn_sb = persist.tile([128, 2, NT, 128], bf16, tag="kn")
        qn_sb = persist.tile([128, 2, NT, 128], bf16, tag="qn")
        kss = persist.tile([128, NT, 4], f32, tag="kss")
        qss = persist.tile([128, NT, 4], f32, tag="qss")
        krs = persist.tile([128, NT, 4], f32, tag="krs")
        qrs = persist.tile([128, NT, 4], f32, tag="qrs")
        knr = persist.tile([128, NT, 4], f32, tag="knr")
        qnr = persist.tile([128, NT, 4], f32, tag="qnr")
        kt_sb = persist.tile([128, 2, S], bf16, tag="kt")
        qt_sb = persist.tile([128, 2, S], bf16, tag="qt")

        # ---- input DMAs: weights, then x/ke streamed by s-range ----
        NSR = 4                       # s-ranges for streaming
        SRW = S // NSR
        nc.sync.dma_start(out=wvk_sb, in_=wvkT_d.rearrange("(c p) m -> p c m", p=128))
        nc.sync.dma_start(out=x_sb[:, :, 0:SRW], in_=xT_r[:, :, 0:SRW])
        nc.sync.dma_start(out=wq_sb, in_=wqT_d.rearrange("(c p) m -> p c m", p=128))
        nc.sync.dma_start(out=ke_sb[:, :, 0:SRW], in_=keT_r[:, :, 0:SRW])
        nc.sync.dma_start(out=mask_sb, in_=mask_d[:, :])
        for sr in range(1, NSR):
            sl = slice(sr * SRW, (sr + 1) * SRW)
            nc.sync.dma_start(out=x_sb[:, :, sl], in_=xT_r[:, :, sl])
            nc.sync.dma_start(out=ke_sb[:, :, sl], in_=keT_r[:, :, sl])

        nc.gpsimd.memset(v_sb[:, :, :, 64], 1.0)

        proj_ctx = ExitStack()
        pvkp = proj_ctx.enter_context(
            tc.tile_pool(name=f"pvk_ps{_rep}", bufs=3, space="PSUM"))
        pqp = proj_ctx.enter_context(
            tc.tile_pool(name=f"pq_ps{_rep}", bufs=3, space="PSUM"))
        wup = proj_ctx.enter_context(
            tc.tile_pool(name=f"wu_ps{_rep}", bufs=1, space="PSUM"))

        # warm the PE clock (HAM) with dummy matmuls while input DMA streams
        WARMUP = True
        if WARMUP:
            wsrc = persist.tile([128, 512], bf16, tag="wsrc")
            nc.vector.memset(wsrc[:, :], 0.0)
            wps = wup.tile([128, 512], f32, tag="wps", name="wps")
            for _ in range(8):
                nc.tensor.matmul(wps[:, :], lhsT=wsrc[:, 0:128],
                                 rhs=wsrc[:, :], start=True, stop=True)

        def half_norms(half):
            """sqrt+recip for a half, then normalize muls + batched
            transposes for that half's tiles."""
            hs = slice(half * NH, (half + 1) * NH)
            # rsqrt = exp(-0.5*ln(x)): keeps ACT on one table (ln/exp/copy)
            nc.scalar.activation(knr[:, hs, :], kss[:, hs, :], AF.Ln)
            nc.scalar.activation(krs[:, hs, :], knr[:, hs, :], AF.Exp,
                                 scale=-0.5)
            nc.scalar.activation(qnr[:, hs, :], qss[:, hs, :], AF.Ln)
            nc.scalar.activation(qrs[:, hs, :], qnr[:, hs, :], AF.Exp,
                                 scale=-0.5)
            ngrp = 2 if half == 0 else 1      # quarters early, then halves
            for gi in range(ngrp):
                t0 = half * NH + gi * (NH // ngrp)
                t1 = t0 + NH // ngrp
                gsl = slice(t0, t1)
                ssl = slice(t0 * 128, t1 * 128)
                for pair in range(2):
                    for st_i in range(t0, t1) if stage >= 4 else []:
                        for hh in range(2):
                            h4 = pair * 2 + hh
                            hsl = slice(hh * 64, (hh + 1) * 64)
                            nc.vector.tensor_scalar_mul(
                                kn_sb[:, pair, st_i, hsl],
                                kn_sb[:, pair, st_i, hsl],
                                krs[:, st_i, h4:h4 + 1])
                            nc.vector.tensor_scalar_mul(
                                qn_sb[:, pair, st_i, hsl],
                                qn_sb[:, pair, st_i, hsl],
                                qrs[:, st_i, h4:h4 + 1])
                    if stage >= 5:
                        nc.sync.dma_start_transpose(
                            kt_sb[:, pair, ssl].rearrange(
                                "p (t c) -> p t c", c=128),
                            kn_sb[:, pair, gsl, :])
                        nc.sync.dma_start_transpose(
                            qt_sb[:, pair, ssl].rearrange(
                                "p (t c) -> p t c", c=128),
                            qn_sb[:, pair, gsl, :])

        # ---- projections ----
        for st_i in range(NT):
            ssl = slice(st_i * 128, (st_i + 1) * 128)
            psvk = pvkp.tile([128, 512], f32, tag="pvk", name="pvk")
            for ec in range(NE):
                nc.tensor.matmul(
                    psvk[:, :], lhsT=x_sb[:, ec, ssl], rhs=wvk_sb[:, ec, :],
                    start=(ec == 0), stop=(ec == NE - 1))
            ccopy = nc.scalar.copy
            ccopy(
                v_sb[:, st_i, :, 0:64],
                psvk[:, 0:256].rearrange("p (h d) -> p h d", h=4))
            ccopy(
                kn_sb[:, :, st_i, :],
                psvk[:, 256:512].rearrange("p (r d) -> p r d", r=2))

            psq = pqp.tile([128, 256], f32, tag="pq", name="pq")
            for ec in range(NE):
                nc.tensor.matmul(
                    psq[:, :], lhsT=ke_sb[:, ec, ssl], rhs=wq_sb[:, ec, :],
                    start=(ec == 0), stop=(ec == NE - 1))
            ccopy(
                qn_sb[:, :, st_i, :],
                psq[:, :].rearrange("p (r d) -> p r d", r=2))

            if stage < 2:
                continue
            # squared norms: square on GPSIMD, reduce on DVE
            # (tensor_tensor_reduce is broken on HW)
            ksq = sqp.tile([128, 256], bf16, tag="ksq", name="ksq")
            nc.gpsimd.tensor_mul(
                ksq[:, :].rearrange("p (r d) -> p r d", r=2),
                kn_sb[:, :, st_i, :], kn_sb[:, :, st_i, :])
            nc.vector.tensor_reduce(
                kss[:, st_i, :], ksq[:, :].rearrange("p (h d) -> p h d", h=4),
                axis=mybir.AxisListType.X, op=ALU.add)
            qsq = sqp.tile([128, 256], bf16, tag="qsq", name="qsq")
            nc.gpsimd.tensor_mul(
                qsq[:, :].rearrange("p (r d) -> p r d", r=2),
                qn_sb[:, :, st_i, :], qn_sb[:, :, st_i, :])
            nc.vector.tensor_reduce(
                qss[:, st_i, :], qsq[:, :].rearrange("p (h d) -> p h d", h=4),
                axis=mybir.AxisListType.X, op=ALU.add)

            if st_i == NH - 1 and stage >= 3:
                half_norms(0)
        if stage >= 3:
            half_norms(1)
        if stage < 7:
            dot = osb.tile([128, NC, 65], f32, tag="ot", name="ot")
            nc.vector.memset(dot[:, :, :], 0.0)
            nc.sync.dma_start(out=out_r[0, :, :, 0, :], in_=dot[:, :, :])
        proj_ctx.close()

        # ---- attention: 16 (j-block, head) units, software-pipelined so
        # block b+1's scores/exp are emitted ahead of block b's PV — the
        # ACT exp stream never waits on PV at block boundaries ----
        with tc.tile_pool(name=f"st_ps{_rep}", bufs=2, space="PSUM") as stp, \
             tc.tile_pool(name=f"pv_ps{_rep}", bufs=2, space="PSUM") as pvp:
            def attn_scores(j, h):
                """scores + exp + mask for block (j, h); returns et buffer."""
                pair, hp = h // 2, h % 2
                off = hp * 64
                ni = 4 * j + 4
                etb = etp.tile([128, NT, SQB], bf16, tag="et", name="et")
                for i0 in range(0, ni, 2):
                    co0 = max(0, 128 * i0 - SQB * j)
                    stps = stp.tile([128, 2, SQB], f32, tag="st", name="st")
                    for ii in range(2):
                        i = i0 + ii
                        ksl = slice(i * 128, (i + 1) * 128)
                        # write from co0 (pair min) so the merged exp
                        # below reads only initialized PSUM
                        nc.tensor.matmul(
                            stps[:, ii, co0:SQB],
                            lhsT=kt_sb[off:off + 64, pair, ksl],
                            rhs=qt_sb[off:off + 64, pair,
                                      j * SQB + co0:(j + 1) * SQB],
                            start=True, stop=True)
                    nc.scalar.activation(
                        etb[:, i0:i0 + 2, co0:SQB],
                        stps[:, :, co0:SQB], AF.Exp, scale=float(g))
                    for ii in range(2):
                        i = i0 + ii
                        if 128 * i >= SQB * j:       # diagonal: causal mask
                            co = 128 * i - SQB * j
                            nc.gpsimd.tensor_mul(
                                etb[:, i, co:co + 128],
                                etb[:, i, co:co + 128], mask_sb[:, :])
                return etb

            def attn_pv(j, h, etb):
                """PV c-outer/i-inner; stage in SBUF, DMA per (j,h)."""
                ot = osb.tile([128, NC, 65], f32, tag="ot", name="ot")
                for c in range(NC):
                    pv = pvp.tile([128, 512], f32, tag="pv", name="pv")
                    last = 4 * j + c
                    csl = slice(c * 128, (c + 1) * 128)
                    for i in range(last + 1):
                        nc.tensor.matmul(
                            pv[:, 0:65],
                            lhsT=etb[:, i, csl],
                            rhs=v_sb[:, i, h, :],
                            start=(i == 0), stop=(i == last))
                    nc.vector.tensor_copy(ot[:, c, :], pv[:, 0:65])
                nc.sync.dma_start(
                    out=out_r[j, :, :, h, :], in_=ot[:, :, :])

            blocks = [(j, h) for j in range(NJ) for h in range(4)]
            if stage == 6:                    # scores/exp only, no PV
                for j, h in blocks:
                    attn_scores(j, h)
            elif stage >= 7:
                pending = None
                for j, h in blocks:
                    etb = attn_scores(j, h)
                    if pending is not None:
                        attn_pv(*pending)
                    pending = (j, h, etb)
                attn_pv(*pending)
    nc.compile()
    # late-added event-sem waits can leave matmuls with 2 sync waits, which
    # walrus cannot encode; re-running this pass hoists extras onto ldweights
    nc.move_matmul_waits_to_ldweights()
    return nc


_NC_CACHE = {}


def _get_nc(g: float, repeats: int = 1):
    key = (g, repeats)
    if key not in _NC_CACHE:
        _NC_CACHE[key] = _build(g, repeats)
    return _NC_CACHE[key]


def _numpy_fallback(x, k_embed, attn_mask, key_padding_mask, Wq, Wk, Wv, g_scale):
    def l2n(t):
        n = np.sqrt((t * t).sum(-1, keepdims=True))
        return t / np.maximum(n, 1e-12)
    q = (k_embed @ Wq.T).reshape(B, S, H, Dh).transpose(0, 2, 1, 3)
    k = (x @ Wk.T).reshape(B, S, H, Dh).transpose(0, 2, 1, 3)
    v = (x @ Wv.T).reshape(B, S, H, Dh).transpose(0, 2, 1, 3)
    q, k = l2n(q), l2n(k)
    s = float(g_scale) * np.einsum('bhqd,bhkd->bhqk', q, k)
    s = np.where(attn_mask[None, None], -np.inf, s)
    s = np.where(key_padding_mask[:, None, None, :], -np.inf, s)
    s = s - s.max(-1, keepdims=True)
    e = np.exp(s)
    a = e / e.sum(-1, keepdims=True)
    o = np.einsum('bhqk,bhkd->bhqd', a, v)
    return o.transpose(0, 2, 1, 3).reshape(B, S, E).astype(np.float32)


def _make_in_maps(x, k_embed, Wq, Wk, Wv):
    import ml_dtypes
    bf = ml_dtypes.bfloat16
    mask01 = np.triu(np.ones((128, 128), np.float32)).astype(bf)  # keep sq>=sk
    in_maps = []
    for c in range(8):
        b, hg = c // 2, c % 2
        rows = slice(hg * 256, (hg + 1) * 256)
        wv_t = Wv[rows].T                       # [512, 256]
        wk_t = Wk[rows].T
        in_maps.append({
            "xt": np.ascontiguousarray(x[b].T).astype(bf),
            "ket": np.ascontiguousarray(k_embed[b].T).astype(bf),
            "wqt": np.ascontiguousarray(Wq[rows].T).astype(bf),
            "wvkt": np.ascontiguousarray(
                np.concatenate([wv_t, wk_t], axis=1)).astype(bf),
            "mask01": mask01,
        })
    return in_maps


def kernel(**inputs) -> np.ndarray:
    x = np.asarray(inputs["x"], np.float32)
    k_embed = np.asarray(inputs["k_embed"], np.float32)
    attn_mask = np.asarray(inputs["attn_mask"])
    key_padding_mask = np.asarray(inputs["key_padding_mask"])
    Wq = np.asarray(inputs["Wq"], np.float32)
    Wk = np.asarray(inputs["Wk"], np.float32)
    Wv = np.asarray(inputs["Wv"], np.float32)
    g = float(np.asarray(inputs["g_scale"]))

    causal = np.triu(np.ones((S, S), bool), k=1)
    if (attn_mask != causal).any() or key_padding_mask.any():
        return _numpy_fallback(x, k_embed, attn_mask, key_padding_mask,
                               Wq, Wk, Wv, g)

    from concourse.bass_utils import run_bass_kernel_spmd

    nc = _get_nc(g)
    in_maps = _make_in_maps(x, k_embed, Wq, Wk, Wv)
    res = run_bass_kernel_spmd(nc, in_maps, core_ids=list(range(8)))
    kernel._last_results = res

    out = np.empty((B, S, E), np.float32)
    for c in range(8):
        b, hg = c // 2, c % 2
        r = res.results[c]["outn"]             # [S, 4, 65]
        w = r[:, :, 0:64] / r[:, :, 64:65]     # softmax denominator
        out[b, :, hg * 256:(hg + 1) * 256] = w.reshape(S, 256)
    return out


# revision 28
# speedup vs baseline: 1.0564x; 1.0564x over previous
"""Trainium2 Bass kernel for causal cross-attention with L2-normalized q/k.

Reference computation (B=4, S=2048, E=512, H=8, Dh=64):
    q = k_embed @ Wq.T ; k = x @ Wk.T ; v = x @ Wv.T        (per batch)
    q,k l2-normalized over Dh per head; scores = g * q @ k.T (causal mask)
    out = softmax(scores) @ v

Sharding: 8 cores = 4 batches x 2 head-groups (4 heads each).

Per-core pipeline (all matmuls bf16, PSUM accumulation fp32):
  - inputs DMA'd as bf16 (host casts), streamed in s-range chunks so
    projections start early
  - projections in natural layout, kn/qn stored pair-major; v copied to SBUF
    by ACT, kn/qn by DVE; squared norms via fused DVE tensor_tensor_reduce;
    sqrt on ACT per half; normalize muls split DVE (pair 0) / GPSIMD (pair 1);
    q/k transposed with batched DMA-engine xbar transposes (8 total)
  - attention per (j-block of 512 sq, head): scores transposed st[sk, sq]
    (K=64), two k-tiles per PSUM tile; one merged exp (ACT, scale=g) per
    i-pair into an SBUF et block buffer (bf16); causal diagonal via 128x128
    keep-mask mul (DVE)
  - PV in natural layout, c-outer/i-inner: lhsT = et 128x128 chunk
    (stationary), moving operand = v (65 cols: 64 v-cols + ones col for the
    softmax denominator); accumulated in a 1-bank PSUM tile [sq_chunk, 65]
  - out staged via DVE to SBUF, DMA'd as [S, 4, 65]; host divides by the
    denominator column and concatenates slices.
"""

import numpy as np

B, S, E, H = 4, 2048, 512, 8
Dh = 64
NE = E // 128           # 4 contraction chunks
NT = S // 128           # 16 s-tiles
SQB = 512               # sq block width
NJ = S // SQB           # 4
NC = SQB // 128         # 4 sq chunks per block
NH = NT // 2            # tiles per half


def _build(g: float, repeats: int = 1, stage: int = 7):
    from contextlib import ExitStack

    import concourse.tile as tile
    from concourse import bacc, mybir

    f32 = mybir.dt.float32
    bf16 = mybir.dt.bfloat16
    AF = mybir.ActivationFunctionType
    ALU = mybir.AluOpType

    nc = bacc.Bacc("TRN2", target_bir_lowering=False, debug=False)
    # Pin the ACT function-set choice to a table containing every function
    # this kernel uses (copy/ln/exp), so exactly one table load is emitted.
    import types

    import bass_rust as _br
    from concourse.hw_specs import get_activation_tables

    def _insert_act_table_loads(self):
        has_activation = any(
            isinstance(i, mybir.InstActivation)
            for b in self.main_func.blocks
            for i in b.instructions
        )
        if not has_activation:
            return
        tables = list(get_activation_tables(self.m.arch).items())
        need = {mybir.ActivationFunctionType.Copy,
                mybir.ActivationFunctionType.Ln,
                mybir.ActivationFunctionType.Exp}
        # act_func_set_id indexes the ORIGINAL act_info.json order, so keep
        # list positions and instead blank out every non-preferred table;
        # the pass then has a single candidate serving copy/ln/exp.
        tables = [(name, (s if need <= s else set())) for name, s in tables]
        _br.insert_act_table_loads(self, tables)

    nc.insert_act_table_loads = types.MethodType(_insert_act_table_loads, nc)
    xT_d = nc.dram_tensor("xt", [E, S], bf16, kind="ExternalInput")
    keT_d = nc.dram_tensor("ket", [E, S], bf16, kind="ExternalInput")
    wqT_d = nc.dram_tensor("wqt", [E, 256], bf16, kind="ExternalInput")
    wvkT_d = nc.dram_tensor("wvkt", [E, 512], bf16, kind="ExternalInput")
    mask_d = nc.dram_tensor("mask01", [128, 128], bf16, kind="ExternalInput")
    out_d = nc.dram_tensor("outn", [S, 4, 65], f32, kind="ExternalOutput")

    xT_r = xT_d.rearrange("(c p) m -> p c m", p=128)
    keT_r = keT_d.rearrange("(c p) m -> p c m", p=128)
    out_r = out_d.rearrange("(j c p) h e -> j p c h e", j=NJ, c=NC, p=128)

    with tile.TileContext(nc) as tc:
     for _rep in range(repeats):
      with ExitStack() as ctx:
        persist = ctx.enter_context(tc.tile_pool(name=f"persist{_rep}", bufs=1))
        sqp = ctx.enter_context(tc.tile_pool(name=f"sqp{_rep}", bufs=2))
        etp = ctx.enter_context(tc.tile_pool(name=f"etp{_rep}", bufs=3))
        osb = ctx.enter_context(tc.tile_pool(name=f"osb{_rep}", bufs=3))

        # ---- persistent SBUF ----
        wq_sb = persist.tile([128, NE, 256], bf16, tag="wq")
        wvk_sb = persist.tile([128, NE, 512], bf16, tag="wvk")
        mask_sb = persist.tile([128, 128], bf16, tag="mask")
        x_sb = persist.tile([128, NE, S], bf16, tag="x")
        ke_sb = persist.tile([128, NE, S], bf16, tag="ke")
        v_sb = persist.tile([128, NT, 4, 65], bf16, tag="v")
        # pair-major: [s_local, pair, s_tile, dh(2 heads x 64)]
        kn_sb = persist.tile([128, 2, NT, 128], bf16, tag="kn")
        qn_sb = persist.tile([128, 2, NT, 128], bf16, tag="qn")
        kss = persist.tile([128, NT, 4], f32, tag="kss")
        qss = persist.tile([128, NT, 4], f32, tag="qss")
        krs = persist.tile([128, NT, 4], f32, tag="krs")
        qrs = persist.tile([128, NT, 4], f32, tag="qrs")
        knr = persist.tile([128, NT, 4], f32, tag="knr")
        qnr = persist.tile([128, NT, 4], f32, tag="qnr")
        kt_sb = persist.tile([128, 2, S], bf16, tag="kt")
        qt_sb = persist.tile([128, 2, S], bf16, tag="qt")

        # ---- input DMAs: weights, then x/ke streamed by s-range ----
        NSR = 4                       # s-ranges for streaming
        SRW = S // NSR
        nc.sync.dma_start(out=wvk_sb, in_=wvkT_d.rearrange("(c p) m -> p c m", p=128))
        nc.sync.dma_start(out=x_sb[:, :, 0:SRW], in_=xT_r[:, :, 0:SRW])
        nc.sync.dma_start(out=wq_sb, in_=wqT_d.rearrange("(c p) m -> p c m", p=128))
        nc.sync.dma_start(out=ke_sb[:, :, 0:SRW], in_=keT_r[:, :, 0:SRW])
        nc.sync.dma_start(out=mask_sb, in_=mask_d[:, :])
        for sr in range(1, NSR):
            sl = slice(sr * SRW, (sr + 1) * SRW)
            nc.sync.dma_start(out=x_sb[:, :, sl], in_=xT_r[:, :, sl])
            nc.sync.dma_start(out=ke_sb[:, :, sl], in_=keT_r[:, :, sl])

        nc.gpsimd.memset(v_sb[:, :, :, 64], 1.0)

        proj_ctx = ExitStack()
        pvkp = proj_ctx.enter_context(
            tc.tile_pool(name=f"pvk_ps{_rep}", bufs=3, space="PSUM"))
        pqp = proj_ctx.enter_context(
            tc.tile_pool(name=f"pq_ps{_rep}", bufs=3, space="PSUM"))
        wup = proj_ctx.enter_context(
            tc.tile_pool(name=f"wu_ps{_rep}", bufs=1, space="PSUM"))

        # warm the PE clock (HAM) with dummy matmuls while input DMA streams
        WARMUP = True
        if WARMUP:
            wsrc = persist.tile([128, 512], bf16, tag="wsrc")
            nc.vector.memset(wsrc[:, :], 0.0)
            wps = wup.tile([128, 512], f32, tag="wps", name="wps")
            for _ in range(8):
                nc.tensor.matmul(wps[:, :], lhsT=wsrc[:, 0:128],
                                 rhs=wsrc[:, :], start=True, stop=True)

        def half_norms(half):
            """sqrt+recip for a half, then normalize muls + batched
            transposes for that half's tiles."""
            hs = slice(half * NH, (half + 1) * NH)
            # rsqrt = exp(-0.5*ln(x)): keeps ACT on one table (ln/exp/copy)
            nc.scalar.activation(knr[:, hs, :], kss[:, hs, :], AF.Ln)
            nc.scalar.activation(krs[:, hs, :], knr[:, hs, :], AF.Exp,
                                 scale=-0.5)
            nc.scalar.activation(qnr[:, hs, :], qss[:, hs, :], AF.Ln)
            nc.scalar.activation(qrs[:, hs, :], qnr[:, hs, :], AF.Exp,
                                 scale=-0.5)
            ngrp = 2 if half == 0 else 1      # quarters early, then halves
            for gi in range(ngrp):
                t0 = half * NH + gi * (NH // ngrp)
                t1 = t0 + NH // ngrp
                gsl = slice(t0, t1)
                ssl = slice(t0 * 128, t1 * 128)
                for pair in range(2):
                    for st_i in range(t0, t1) if stage >= 4 else []:
                        for hh in range(2):
                            h4 = pair * 2 + hh
                            hsl = slice(hh * 64, (hh + 1) * 64)
                            nc.vector.tensor_scalar_mul(
                                kn_sb[:, pair, st_i, hsl],
                                kn_sb[:, pair, st_i, hsl],
                                krs[:, st_i, h4:h4 + 1])
                            nc.vector.tensor_scalar_mul(
                                qn_sb[:, pair, st_i, hsl],
                                qn_sb[:, pair, st_i, hsl],
                                qrs[:, st_i, h4:h4 + 1])
                    if stage >= 5:
                        nc.sync.dma_start_transpose(
                            kt_sb[:, pair, ssl].rearrange(
                                "p (t c) -> p t c", c=128),
                            kn_sb[:, pair, gsl, :])
                        nc.sync.dma_start_transpose(
                            qt_sb[:, pair, ssl].rearrange(
                                "p (t c) -> p t c", c=128),
                            qn_sb[:, pair, gsl, :])

        # ---- projections ----
        for st_i in range(NT):
            ssl = slice(st_i * 128, (st_i + 1) * 128)
            psvk = pvkp.tile([128, 512], f32, tag="pvk", name="pvk")
            for ec in range(NE):
                nc.tensor.matmul(
                    psvk[:, :], lhsT=x_sb[:, ec, ssl], rhs=wvk_sb[:, ec, :],
                    start=(ec == 0), stop=(ec == NE - 1))
            ccopy = nc.scalar.copy
            ccopy(
                v_sb[:, st_i, :, 0:64],
                psvk[:, 0:256].rearrange("p (h d) -> p h d", h=4))
            ccopy(
                kn_sb[:, :, st_i, :],
                psvk[:, 256:512].rearrange("p (r d) -> p r d", r=2))

            psq = pqp.tile([128, 256], f32, tag="pq", name="pq")
            for ec in range(NE):
                nc.tensor.matmul(
                    psq[:, :], lhsT=ke_sb[:, ec, ssl], rhs=wq_sb[:, ec, :],
                    start=(ec == 0), stop=(ec == NE - 1))
            ccopy(
                qn_sb[:, :, st_i, :],
                psq[:, :].rearrange("p (r d) -> p r d", r=2))

            if stage < 2:
                continue
            # squared norms: square on GPSIMD, reduce on DVE
            # (tensor_tensor_reduce is broken on HW)
            ksq = sqp.tile([128, 256], bf16, tag="ksq", name="ksq")
            nc.gpsimd.tensor_mul(
                ksq[:, :].rearrange("p (r d) -> p r d", r=2),
                kn_sb[:, :, st_i, :], kn_sb[:, :, st_i, :])
            nc.vector.tensor_reduce(
                kss[:, st_i, :], ksq[:, :].rearrange("p (h d) -> p h d", h=4),
                axis=mybir.AxisListType.X, op=ALU.add)
            qsq = sqp.tile([128, 256], bf16, tag="qsq", name="qsq")
            nc.gpsimd.tensor_mul(
                qsq[:, :].rearrange("p (r d) -> p r d", r=2),
                qn_sb[:, :, st_i, :], qn_sb[:, :, st_i, :])
            nc.vector.tensor_reduce(
                qss[:, st_i, :], qsq[:, :].rearrange("p (h d) -> p h d", h=4),
                axis=mybir.AxisListType.X, op=ALU.add)

            if st_i == NH - 1 and stage >= 3:
                half_norms(0)
        if stage >= 3:
            half_norms(1)
        if stage < 7:
            dot = osb.tile([128, NC, 65], f32, tag="ot", name="ot")
            nc.vector.memset(dot[:, :, :], 0.0)
            nc.sync.dma_start(out=out_r[0, :, :, 0, :], in_=dot[:, :, :])
        proj_ctx.close()

        # ---- attention: 16 (j-block, head) units, software-pipelined so
        # block b+1's scores/exp are emitted ahead of block b's PV — the
        # ACT exp stream never waits on PV at block boundaries ----
        with tc.tile_pool(name=f"st_ps{_rep}", bufs=3, space="PSUM") as stp, \
             tc.tile_pool(name=f"pv_ps{_rep}", bufs=2, space="PSUM") as pvp:
            def attn_scores(j, h):
                """scores + exp + mask for block (j, h); returns et buffer."""
                pair, hp = h // 2, h % 2
                off = hp * 64
                ni = 4 * j + 4
                etb = etp.tile([128, NT, SQB], bf16, tag="et", name="et")
                for i0 in range(0, ni, 2):
                    co0 = max(0, 128 * i0 - SQB * j)
                    stps = stp.tile([128, 2, SQB], f32, tag="st", name="st")
                    for ii in range(2):
                        i = i0 + ii
                        ksl = slice(i * 128, (i + 1) * 128)
                        # write from co0 (pair min) so the merged exp
                        # below reads only initialized PSUM
                        nc.tensor.matmul(
                            stps[:, ii, co0:SQB],
                            lhsT=kt_sb[off:off + 64, pair, ksl],
                            rhs=qt_sb[off:off + 64, pair,
                                      j * SQB + co0:(j + 1) * SQB],
                            start=True, stop=True)
                    nc.scalar.activation(
                        etb[:, i0:i0 + 2, co0:SQB],
                        stps[:, :, co0:SQB], AF.Exp, scale=float(g))
                    for ii in range(2):
                        i = i0 + ii
                        if 128 * i >= SQB * j:       # diagonal: causal mask
                            co = 128 * i - SQB * j
                            nc.gpsimd.tensor_mul(
                                etb[:, i, co:co + 128],
                                etb[:, i, co:co + 128], mask_sb[:, :])
                return etb

            def attn_pv(j, h, etb):
                """PV c-outer/i-inner; stage in SBUF, DMA per (j,h)."""
                ot = osb.tile([128, NC, 65], f32, tag="ot", name="ot")
                for c in range(NC):
                    pv = pvp.tile([128, 512], f32, tag="pv", name="pv")
                    last = 4 * j + c
                    csl = slice(c * 128, (c + 1) * 128)
                    for i in range(last + 1):
                        nc.tensor.matmul(
                            pv[:, 0:65],
                            lhsT=etb[:, i, csl],
                            rhs=v_sb[:, i, h, :],
                            start=(i == 0), stop=(i == last))
                    nc.vector.tensor_copy(ot[:, c, :], pv[:, 0:65])
                nc.sync.dma_start(
                    out=out_r[j, :, :, h, :], in_=ot[:, :, :])

            blocks = [(j, h) for j in range(NJ) for h in range(4)]
            if stage == 6:                    # scores/exp only, no PV
                for j, h in blocks:
                    attn_scores(j, h)
            elif stage >= 7:
                pending = None
                for j, h in blocks:
                    etb = attn_scores(j, h)
                    if pending is not None:
                        attn_pv(*pending)
                    pending = (j, h, etb)
                attn_pv(*pending)
    nc.compile()
    # late-added event-sem waits can leave matmuls with 2 sync waits, which
    # walrus cannot encode; re-running this pass hoists extras onto ldweights
    nc.move_matmul_waits_to_ldweights()
    return nc


_NC_CACHE = {}


def _get_nc(g: float, repeats: int = 1):
    key = (g, repeats)
    if key not in _NC_CACHE:
        _NC_CACHE[key] = _build(g, repeats)
    return _NC_CACHE[key]


def _numpy_fallback(x, k_embed, attn_mask, key_padding_mask, Wq, Wk, Wv, g_scale):
    def l2n(t):
        n = np.sqrt((t * t).sum(-1, keepdims=True))
        return t / np.maximum(n, 1e-12)
    q = (k_embed @ Wq.T).reshape(B, S, H, Dh).transpose(0, 2, 1, 3)
    k = (x @ Wk.T).reshape(B, S, H, Dh).transpose(0, 2, 1, 3)
    v = (x @ Wv.T).reshape(B, S, H, Dh).transpose(0, 2, 1, 3)
    q, k = l2n(q), l2n(k)
    s = float(g_scale) * np.einsum('bhqd,bhkd->bhqk', q, k)
    s = np.where(attn_mask[None, None], -np.inf, s)
    s = np.where(key_padding_mask[:, None, None, :], -np.inf, s)
    s = s - s.max(-1, keepdims=True)
    e = np.exp(s)
    a = e / e.sum(-1, keepdims=True)
    o = np.einsum('bhqk,bhkd->bhqd', a, v)
    return o.transpose(0, 2, 1, 3).reshape(B, S, E).astype(np.float32)


def _make_in_maps(x, k_embed, Wq, Wk, Wv):
    import ml_dtypes
    bf = ml_dtypes.bfloat16
    mask01 = np.triu(np.ones((128, 128), np.float32)).astype(bf)  # keep sq>=sk
    in_maps = []
    for c in range(8):
        b, hg = c // 2, c % 2
        rows = slice(hg * 256, (hg + 1) * 256)
        wv_t = Wv[rows].T                       # [512, 256]
        wk_t = Wk[rows].T
        in_maps.append({
            "xt": np.ascontiguousarray(x[b].T).astype(bf),
            "ket": np.ascontiguousarray(k_embed[b].T).astype(bf),
            "wqt": np.ascontiguousarray(Wq[rows].T).astype(bf),
            "wvkt": np.ascontiguousarray(
                np.concatenate([wv_t, wk_t], axis=1)).astype(bf),
            "mask01": mask01,
        })
    return in_maps


def kernel(**inputs) -> np.ndarray:
    x = np.asarray(inputs["x"], np.float32)
    k_embed = np.asarray(inputs["k_embed"], np.float32)
    attn_mask = np.asarray(inputs["attn_mask"])
    key_padding_mask = np.asarray(inputs["key_padding_mask"])
    Wq = np.asarray(inputs["Wq"], np.float32)
    Wk = np.asarray(inputs["Wk"], np.float32)
    Wv = np.asarray(inputs["Wv"], np.float32)
    g = float(np.asarray(inputs["g_scale"]))

    causal = np.triu(np.ones((S, S), bool), k=1)
    if (attn_mask != causal).any() or key_padding_mask.any():
        return _numpy_fallback(x, k_embed, attn_mask, key_padding_mask,
                               Wq, Wk, Wv, g)

    from concourse.bass_utils import run_bass_kernel_spmd

    nc = _get_nc(g)
    in_maps = _make_in_maps(x, k_embed, Wq, Wk, Wv)
    res = run_bass_kernel_spmd(nc, in_maps, core_ids=list(range(8)))
    kernel._last_results = res

    out = np.empty((B, S, E), np.float32)
    for c in range(8):
        b, hg = c // 2, c % 2
        r = res.results[c]["outn"]             # [S, 4, 65]
        w = r[:, :, 0:64] / r[:, :, 64:65]     # softmax denominator
        out[b, :, hg * 256:(hg + 1) * 256] = w.reshape(S, 256)
    return out
